# revision 9
# baseline (speedup 1.0000x reference)
"""Trainium2 Bass kernel for nn_Colorizer (retrieval_knn).

Pipeline (per sample, data-parallel over N=8 samples -> 8 cores):
  1. Patch-embed conv as matmul: featsT[c, p] = W[k, c]^T @ patchesT[k, p]
     (k = 8*8*3 = 192 patch pixels, p = 4 images * 32*32 patches = 4096)
  2. Similarity S[r, t] = refT[c, r]^T @ tgtT[c, t]   (r = 3072, t = 1024)
  3. E = exp(S - 50)  (softmax over r is shift-invariant; max|S| ~= 87 so
     the constant shift prevents fp32 exp overflow; underflow to 0 is safe)
  4. predT_unnorm = labels_aug^T @ E with labels_aug = [ones(16),
     zeros(16), labels(16)] baked host-side: rows 0..15 = replicated
     softmax denominator, rows 32..47 = unnormalized predictions (zeros
     keep the blocks 32-partition-aligned; custom-DVE reciprocal
     requires partition base 0, standard ops handle base 32)
  5. Normalize: out = pred_rows * reciprocal(denom_rows), DMA out as
     [16, 1024]; host transposes to [1024, 16].

Host side only reshapes/transposes/casts data (im2col layout +
sharding + bf16 + constant block concat); all FLOPs run on device.

Perf notes (measured on HW):
  - patches + conv weights ship as bf16: halves the input DMA payload;
    end-to-end rel err ~5e-3, well under the 2e-2 gate. Similarity
    stays f32r (exp amplifies S error; fp8 anywhere in the S path
    fails the gate).
  - input DMA fans out over all 3 DMA paths (sync HWDGE, scalar HWDGE,
    gpsimd SWDGE), one 64-partition row-band each, target block first
    and split per 512 cols so the first conv starts ASAP. The aug-label
    block is a single contiguous DMA split across the two HWDGE rings
    (a strided per-rc DMA would cost 3k tiny SWDGE descriptors).
  - PE warm-up matmuls on uninitialized SBUF (nothing reads the
    result) start right after the engine preamble, so the HAM throttle
    (half rate -> full after ~2.5 us of sustained work) lifts before
    the real matmuls begin. Warm-up tiles come from the shared PSUM
    pool, freeing 2 banks so the matmul pool runs 3-deep (PE can run
    ~3 chunks ahead of the ACT exp stream without stalling).
  - pred matmuls are emitted two chunks behind their exp so the PE
    never waits on ACT (in-order PE queue would otherwise bubble).
  - conv PSUM->SBUF casts run on DVE (+ACT only for the first two
    blocks, before the exp stream starts).
  - the last chunk's exp/pred/normalize/DMA-out are split into two
    512-col halves, with the multiply on gpsimd, so the tail drains
    ~2 us faster.

Built on bacc.Bacc so compile() legalizes multi-semaphore waits (TRN2
instructions accept only one sync wait).
"""

import ml_dtypes
import numpy as np

import concourse.mybir as mybir
from concourse import bacc
from concourse.bass_utils import run_bass_kernel_spmd
from concourse.tile import TileContext

F32 = mybir.dt.float32
F32R = mybir.dt.float32r
BF16 = mybir.dt.bfloat16

N = 8            # samples == cores
R_T, T_T = 3, 1  # ref / target frames
H = W_IMG = 256
C = 3
PATCH = 8
FEAT = 256
K_LAB = 16
HP = H // PATCH          # 32
PPI = HP * HP            # 1024 patches per image
NIMG = R_T + T_T         # 4
NPAT = NIMG * PPI        # 4096
KPIX = PATCH * PATCH * C  # 192
KPAD = 256               # K padded to 2x128 (K=64 matmuls run ~3x slow)
R = R_T * PPI            # 3072
T = T_T * PPI            # 1024
RC = R // 128            # 24 r-chunks
LABC = 48                # 16 ones cols, 16 zero cols, 16 label cols
EXP_SHIFT = -50.0
N_WARMUP = 7  # ~3 us of half-rate warm-up: ends right at the HAM grant


def _build_nc():
    nc = bacc.Bacc(trn_type="TRN2", target_bir_lowering=False)

    pt_d = nc.declare_dram_parameter("pt", [KPIX, NPAT], BF16, isOutput=False)
    w_d = nc.declare_dram_parameter("w", [KPAD, FEAT], BF16, isOutput=False)
    lab_d = nc.declare_dram_parameter("lab", [128, RC * LABC], F32R, isOutput=False)
    out_d = nc.declare_dram_parameter("out", [K_LAB, T], F32, isOutput=True)

    with TileContext(nc) as tc:
        with (
            tc.tile_pool(name="const", bufs=1) as const,
            tc.tile_pool(name="feats", bufs=1) as feats,
            tc.tile_pool(name="mmps", bufs=3, space="PSUM") as mmps,
            tc.tile_pool(name="predps", bufs=1, space="PSUM") as predps,
            tc.tile_pool(name="epool", bufs=4) as epool,
            tc.tile_pool(name="opool", bufs=2) as opool,
        ):
            # PE warm-up source: first DVE op (f32-view memset halves the
            # column count) so the warm-up matmuls can start ASAP
            wu_sb = const.tile([128, 512], BF16, tag="wu")
            nc.vector.memset(wu_sb.bitcast(F32), 0.0)

            shift_sb = const.tile([128, 1], F32, tag="shift")
            nc.vector.memset(shift_sb, EXP_SHIFT)

            # ---- input loads: 3 DMA paths (sync/scalar HWDGE + gpsimd
            # SWDGE), one 64-row band each, tgt image block first ----
            w_sb0 = const.tile([128, FEAT], BF16, tag="w0")
            w_sb1 = const.tile([KPAD - 128, FEAT], BF16, tag="w1")
            nc.sync.dma_start(out=w_sb0, in_=w_d.ap()[0:128, :])
            nc.scalar.dma_start(out=w_sb1, in_=w_d.ap()[128:KPAD, :])

            pt_sb0 = const.tile([128, NPAT], BF16, tag="pt0")
            pt_sb1 = const.tile([KPAD - 128, NPAT], BF16, tag="pt1")
            # pad rows 64..127 of the K=128..255 tile with zeros on-chip
            nc.vector.memset(pt_sb1[64:128, :].bitcast(F32), 0.0)

            lab_sb = const.tile([128, RC, LABC], F32R, tag="lab")
            lab_r = lab_d.ap().rearrange("p (rc k) -> p rc k", k=LABC)

            def pt_load(nb, halves):
                sl = slice(nb * PPI, (nb + 1) * PPI)
                if halves:
                    for hh in range(2):
                        hsl = slice(nb * PPI + hh * 512, nb * PPI + (hh + 1) * 512)
                        nc.sync.dma_start(
                            out=pt_sb0[0:64, hsl], in_=pt_d.ap()[0:64, hsl]
                        )
                        nc.scalar.dma_start(
                            out=pt_sb0[64:128, hsl], in_=pt_d.ap()[64:128, hsl]
                        )
                else:
                    nc.sync.dma_start(out=pt_sb0[0:64, sl], in_=pt_d.ap()[0:64, sl])
                    nc.scalar.dma_start(
                        out=pt_sb0[64:128, sl], in_=pt_d.ap()[64:128, sl]
                    )
                nc.gpsimd.dma_start(
                    out=pt_sb1[0:KPIX - 128, sl], in_=pt_d.ap()[128:KPIX, sl]
                )

            pt_load(3, halves=True)
            pt_load(0, halves=True)
            # aug-labels next: needed by the first pred (~3 chunks in)
            nc.sync.dma_start(out=lab_sb[:, 0:12, :], in_=lab_r[:, 0:12, :])
            nc.scalar.dma_start(out=lab_sb[:, 12:RC, :], in_=lab_r[:, 12:RC, :])
            pt_load(1, halves=False)
            pt_load(2, halves=False)

            # ---- PE clock warm-up during the DMA prologue (HAM) ----
            for _ in range(N_WARMUP):
                wps = mmps.tile([128, 512], F32, tag="mm", name="wps")
                nc.tensor.matmul(wps, wu_sb[:, 0:128], wu_sb, start=True, stop=True)

            # ---- 1. conv: featsT[c, p] (c split in two 128-row tiles) ----
            f_sb = [
                feats.tile([128, NPAT], F32R, tag="f0", name="f_sb0"),
                feats.tile([128, NPAT], F32R, tag="f1", name="f_sb1"),
            ]
            NB = 4  # column blocks of 1024
            BW = NPAT // NB
            pred_ps = predps.tile([LABC, T], F32, tag="pred")

            def conv_block(nb, act_cast):
                for cc in range(2):
                    ps = mmps.tile([128, BW], F32, tag="mm", name="ps")
                    csl = slice(cc * 128, (cc + 1) * 128)
                    for h in range(2):
                        hsl = slice(nb * BW + h * 512, nb * BW + (h + 1) * 512)
                        psl = slice(h * 512, (h + 1) * 512)
                        nc.tensor.matmul(
                            ps[:, psl], w_sb0[:, csl], pt_sb0[:, hsl],
                            start=True, stop=False,
                        )
                        nc.tensor.matmul(
                            ps[:, psl], w_sb1[:, csl], pt_sb1[:, hsl],
                            start=False, stop=True,
                        )
                    dst = f_sb[cc][:, nb * BW:(nb + 1) * BW]
                    # before the exp stream starts ACT is idle: split each
                    # cast DVE/ACT so the S chunks unblock ~1 us sooner;
                    # afterwards DVE (which idles in steady state) takes all
                    if act_cast:
                        nc.vector.tensor_copy(dst[:, 0:512], ps[:, 0:512])
                        nc.scalar.copy(dst[:, 512:BW], ps[:, 512:BW])
                    else:
                        nc.vector.tensor_copy(dst, ps)

            e_tiles = {}

            def s_part(rc):
                rsl = slice(rc * 128, (rc + 1) * 128)
                s_ps = mmps.tile([128, T], F32, tag="mm", name="s_ps")
                for th in range(2):
                    psl = slice(th * 512, (th + 1) * 512)
                    tsl = slice(R + th * 512, R + (th + 1) * 512)
                    nc.tensor.matmul(
                        s_ps[:, psl], f_sb[0][:, rsl], f_sb[0][:, tsl],
                        start=True, stop=False,
                    )
                    nc.tensor.matmul(
                        s_ps[:, psl], f_sb[1][:, rsl], f_sb[1][:, tsl],
                        start=False, stop=True,
                    )
                e_sb = epool.tile([128, T], F32R, tag="e", name="e_sb")
                if rc == RC - 1:
                    # split the last exp so the tail drains per 512-col half
                    for th in range(2):
                        psl = slice(th * 512, (th + 1) * 512)
                        nc.scalar.activation(
                            e_sb[:, psl], s_ps[:, psl],
                            mybir.ActivationFunctionType.Exp,
                            bias=shift_sb, scale=1.0,
                        )
                else:
                    nc.scalar.activation(
                        e_sb, s_ps, mybir.ActivationFunctionType.Exp,
                        bias=shift_sb, scale=1.0,
                    )
                e_tiles[rc] = e_sb

            def pred_part(rc):
                e_sb = e_tiles.pop(rc)
                for th in range(2):
                    psl = slice(th * 512, (th + 1) * 512)
                    nc.tensor.matmul(
                        pred_ps[:, psl],
                        lab_sb[:, rc, :],
                        e_sb[:, psl],
                        start=(rc == 0), stop=(rc == RC - 1),
                    )

            # conv blocks feed S chunks; pred lags two chunks behind its exp
            PRED_LAG = 2
            emitted = []

            def emit_s(rc):
                s_part(rc)
                emitted.append(rc)
                if len(emitted) > PRED_LAG:
                    pred_part(emitted[len(emitted) - 1 - PRED_LAG])

            conv_block(3, act_cast=True)
            conv_block(0, act_cast=True)
            for rc in range(0, 8):
                emit_s(rc)
            conv_block(1, act_cast=False)
            for rc in range(8, 16):
                emit_s(rc)
            conv_block(2, act_cast=False)
            for rc in range(16, 24):
                emit_s(rc)
            for rc in emitted[-PRED_LAG:]:
                pred_part(rc)

            # ---- 5. normalize label rows by replicated denom rows ----
            # per 512-col half across 3 engines: ACT copies the numerators
            # out of PSUM (gpsimd can't read PSUM) while DVE does the
            # reciprocals, then gpsimd multiplies — the halves pipeline
            rec = opool.tile([K_LAB, T], F32, tag="rec")
            num_sb = opool.tile([K_LAB, T], F32, tag="num")
            o_sb = opool.tile([K_LAB, T], F32, tag="o")
            for th in range(2):
                psl = slice(th * 512, (th + 1) * 512)
                nc.scalar.copy(num_sb[:, psl], pred_ps[32:32 + K_LAB, psl])
                nc.vector.reciprocal_approx_fast(
                    rec[:, psl], pred_ps[0:K_LAB, psl]
                )
                nc.gpsimd.tensor_mul(
                    o_sb[:, psl], num_sb[:, psl], rec[:, psl]
                )
                nc.sync.dma_start(out=out_d.ap()[:, psl], in_=o_sb[:, psl])

    nc.compile()
    return nc


_NC_CACHE = None


def _get_nc():
    global _NC_CACHE
    if _NC_CACHE is None:
        _NC_CACHE = _build_nc()
    return _NC_CACHE


def prep_in_maps(reference_images, target_images, reference_labels, w_feat):
    """Host-side sharding + layout prep (no arithmetic)."""
    ri = np.ascontiguousarray(reference_images, dtype=np.float32)
    ti = np.ascontiguousarray(target_images, dtype=np.float32)
    lab = np.ascontiguousarray(reference_labels, dtype=np.float32)
    wf = np.ascontiguousarray(w_feat, dtype=np.float32)

    w2 = np.zeros((KPAD, FEAT), ml_dtypes.bfloat16)
    w2[:KPIX] = wf.reshape(KPIX, FEAT).astype(ml_dtypes.bfloat16)
    imgs = np.concatenate([ri, ti], axis=1)  # [N, 4, H, W, C]
    # patchesT[n] : [(dy dx ch), (img py px)]
    ptT = np.ascontiguousarray(
        imgs.reshape(N, NIMG, HP, PATCH, HP, PATCH, C)
        .transpose(0, 3, 5, 6, 1, 2, 4)
        .reshape(N, KPIX, NPAT)
        .astype(ml_dtypes.bfloat16)
    )
    # aug labels: [128, RC, 48] = [ones16 | zeros16 | lab16] per r-chunk
    lab_sw = np.zeros((N, 128, RC, LABC), np.float32)
    lab_sw[:, :, :, 0:K_LAB] = 1.0
    lab_sw[:, :, :, 32:48] = (
        lab.reshape(N, RC, 128, K_LAB).transpose(0, 2, 1, 3)
    )
    lab_sw = np.ascontiguousarray(lab_sw.reshape(N, 128, RC * LABC))
    return [
        {"pt": ptT[n], "w": w2, "lab": lab_sw[n]} for n in range(N)
    ]


def run(in_maps, **kwargs):
    nc = _get_nc()
    return run_bass_kernel_spmd(nc, in_maps, list(range(N)), **kwargs)


def kernel(reference_images, target_images, reference_labels, w_feat):
    in_maps = prep_in_maps(
        reference_images, target_images, reference_labels, w_feat
    )
    res = run(in_maps)
    # device emits [16, T]; transpose to [T, 16] here (pure layout)
    out = np.stack(
        [np.ascontiguousarray(res.results[n]["out"].T) for n in range(N)]
    )
    return out.reshape(N, T_T, HP, HP, K_LAB)


# revision 11
# speedup vs baseline: 1.0473x; 1.0473x over previous
"""Trainium2 Bass kernel for nn_Colorizer (retrieval_knn).

Pipeline (per sample, data-parallel over N=8 samples -> 8 cores):
  1. Patch-embed conv as matmul: featsT[c, p] = W[k, c]^T @ patchesT[k, p]
     (k = 8*8*3 = 192 patch pixels, p = 4 images * 32*32 patches = 4096)
  2. Similarity S[r, t] = refT[c, r]^T @ tgtT[c, t]   (r = 3072, t = 1024)
  3. E = exp(S - 50)  (softmax over r is shift-invariant; max|S| ~= 87 so
     the constant shift prevents fp32 exp overflow; underflow to 0 is safe)
  4. predT_unnorm = labels_aug^T @ E with labels_aug = [ones(16),
     zeros(16), labels(16)] baked host-side: rows 0..15 = replicated
     softmax denominator, rows 32..47 = unnormalized predictions (zeros
     keep the blocks 32-partition-aligned; custom-DVE reciprocal
     requires partition base 0, standard ops handle base 32)
  5. Normalize: out = pred_rows * reciprocal(denom_rows), DMA out as
     [16, 1024]; host transposes to [1024, 16].

Host side only reshapes/transposes/casts data (im2col layout +
sharding + bf16 + constant block concat); all FLOPs run on device.

Perf notes (measured on HW):
  - patches + conv weights ship as bf16: halves the input DMA payload;
    end-to-end rel err ~5e-3, well under the 2e-2 gate. Similarity
    stays f32r (exp amplifies S error; fp8 anywhere in the S path
    fails the gate).
  - input DMA fans out over all 3 DMA paths (sync HWDGE, scalar HWDGE,
    gpsimd SWDGE), one 64-partition row-band each, target block first
    and split per 512 cols so the first conv starts ASAP. The aug-label
    block is a single contiguous DMA split across the two HWDGE rings
    (a strided per-rc DMA would cost 3k tiny SWDGE descriptors).
  - PE warm-up matmuls on uninitialized SBUF (nothing reads the
    result) start right after the engine preamble, so the HAM throttle
    (half rate -> full after ~2.5 us of sustained work) lifts before
    the real matmuls begin. Warm-up tiles come from the shared PSUM
    pool, freeing 2 banks so the matmul pool runs 3-deep (PE can run
    ~3 chunks ahead of the ACT exp stream without stalling).
  - pred matmuls are emitted two chunks behind their exp so the PE
    never waits on ACT (in-order PE queue would otherwise bubble).
  - conv PSUM->SBUF casts run on DVE (+ACT only for the first two
    blocks, before the exp stream starts).
  - the last chunk's exp/pred/normalize/DMA-out are split into two
    512-col halves so the tail drains ~2 us faster (reciprocal and
    multiply stay on DVE: gpsimd cannot read PSUM).

Built on bacc.Bacc so compile() legalizes multi-semaphore waits (TRN2
instructions accept only one sync wait).
"""

import ml_dtypes
import numpy as np

import concourse.mybir as mybir
from concourse import bacc
from concourse.bass_utils import run_bass_kernel_spmd
from concourse.tile import TileContext

F32 = mybir.dt.float32
F32R = mybir.dt.float32r
BF16 = mybir.dt.bfloat16

N = 8            # samples == cores
R_T, T_T = 3, 1  # ref / target frames
H = W_IMG = 256
C = 3
PATCH = 8
FEAT = 256
K_LAB = 16
HP = H // PATCH          # 32
PPI = HP * HP            # 1024 patches per image
NIMG = R_T + T_T         # 4
NPAT = NIMG * PPI        # 4096
KPIX = PATCH * PATCH * C  # 192
KPAD = 256               # K padded to 2x128 (K=64 matmuls run ~3x slow)
R = R_T * PPI            # 3072
T = T_T * PPI            # 1024
RC = R // 128            # 24 r-chunks
LABC = 48                # 16 ones cols, 16 zero cols, 16 label cols
EXP_SHIFT = -50.0
N_WARMUP = 12


def _build_nc():
    nc = bacc.Bacc(trn_type="TRN2", target_bir_lowering=False)

    pt_d = nc.declare_dram_parameter("pt", [KPIX, NPAT], BF16, isOutput=False)
    w_d = nc.declare_dram_parameter("w", [KPAD, FEAT], BF16, isOutput=False)
    lab_d = nc.declare_dram_parameter("lab", [128, RC * LABC], F32R, isOutput=False)
    out_d = nc.declare_dram_parameter("out", [K_LAB, T], F32, isOutput=True)

    with TileContext(nc) as tc:
        with (
            tc.tile_pool(name="const", bufs=1) as const,
            tc.tile_pool(name="feats", bufs=1) as feats,
            tc.tile_pool(name="mmps", bufs=3, space="PSUM") as mmps,
            tc.tile_pool(name="predps", bufs=1, space="PSUM") as predps,
            tc.tile_pool(name="epool", bufs=4) as epool,
            tc.tile_pool(name="opool", bufs=2) as opool,
        ):
            # PE warm-up source: first DVE op (f32-view memset halves the
            # column count) so the warm-up matmuls can start ASAP
            wu_sb = const.tile([128, 512], BF16, tag="wu")
            nc.vector.memset(wu_sb.bitcast(F32), 0.0)

            shift_sb = const.tile([128, 1], F32, tag="shift")
            nc.vector.memset(shift_sb, EXP_SHIFT)

            # ---- input loads: 3 DMA paths (sync/scalar HWDGE + gpsimd
            # SWDGE), one 64-row band each, tgt image block first ----
            w_sb0 = const.tile([128, FEAT], BF16, tag="w0")
            w_sb1 = const.tile([KPAD - 128, FEAT], BF16, tag="w1")
            nc.sync.dma_start(out=w_sb0, in_=w_d.ap()[0:128, :])
            nc.scalar.dma_start(out=w_sb1, in_=w_d.ap()[128:KPAD, :])

            pt_sb0 = const.tile([128, NPAT], BF16, tag="pt0")
            pt_sb1 = const.tile([KPAD - 128, NPAT], BF16, tag="pt1")
            # pad rows 64..127 of the K=128..255 tile with zeros on-chip
            nc.vector.memset(pt_sb1[64:128, :].bitcast(F32), 0.0)

            lab_sb = const.tile([128, RC, LABC], F32R, tag="lab")
            lab_r = lab_d.ap().rearrange("p (rc k) -> p rc k", k=LABC)

            def pt_load(nb, halves):
                sl = slice(nb * PPI, (nb + 1) * PPI)
                if halves:
                    for hh in range(2):
                        hsl = slice(nb * PPI + hh * 512, nb * PPI + (hh + 1) * 512)
                        nc.sync.dma_start(
                            out=pt_sb0[0:64, hsl], in_=pt_d.ap()[0:64, hsl]
                        )
                        nc.scalar.dma_start(
                            out=pt_sb0[64:128, hsl], in_=pt_d.ap()[64:128, hsl]
                        )
                else:
                    nc.sync.dma_start(out=pt_sb0[0:64, sl], in_=pt_d.ap()[0:64, sl])
                    nc.scalar.dma_start(
                        out=pt_sb0[64:128, sl], in_=pt_d.ap()[64:128, sl]
                    )
                nc.gpsimd.dma_start(
                    out=pt_sb1[0:KPIX - 128, sl], in_=pt_d.ap()[128:KPIX, sl]
                )

            pt_load(3, halves=True)
            pt_load(0, halves=True)
            # aug-labels next: needed by the first pred (~3 chunks in)
            nc.sync.dma_start(out=lab_sb[:, 0:12, :], in_=lab_r[:, 0:12, :])
            nc.scalar.dma_start(out=lab_sb[:, 12:RC, :], in_=lab_r[:, 12:RC, :])
            pt_load(1, halves=False)
            pt_load(2, halves=False)

            # ---- PE clock warm-up during the DMA prologue (HAM) ----
            for _ in range(N_WARMUP):
                wps = mmps.tile([128, 512], F32, tag="mm", name="wps")
                nc.tensor.matmul(wps, wu_sb[:, 0:128], wu_sb, start=True, stop=True)

            # ---- 1. conv: featsT[c, p] (c split in two 128-row tiles) ----
            f_sb = [
                feats.tile([128, NPAT], F32R, tag="f0", name="f_sb0"),
                feats.tile([128, NPAT], F32R, tag="f1", name="f_sb1"),
            ]
            NB = 4  # column blocks of 1024
            BW = NPAT // NB
            pred_ps = predps.tile([LABC, T], F32, tag="pred")

            def conv_block(nb, act_cast):
                for cc in range(2):
                    ps = mmps.tile([128, BW], F32, tag="mm", name="ps")
                    csl = slice(cc * 128, (cc + 1) * 128)
                    for h in range(2):
                        hsl = slice(nb * BW + h * 512, nb * BW + (h + 1) * 512)
                        psl = slice(h * 512, (h + 1) * 512)
                        nc.tensor.matmul(
                            ps[:, psl], w_sb0[:, csl], pt_sb0[:, hsl],
                            start=True, stop=False,
                        )
                        nc.tensor.matmul(
                            ps[:, psl], w_sb1[:, csl], pt_sb1[:, hsl],
                            start=False, stop=True,
                        )
                    dst = f_sb[cc][:, nb * BW:(nb + 1) * BW]
                    # before the exp stream starts ACT is idle: give it
                    # the cc=1 casts; afterwards DVE takes everything
                    if act_cast and cc == 1:
                        nc.scalar.copy(dst, ps)
                    else:
                        nc.vector.tensor_copy(dst, ps)

            e_tiles = {}

            def s_part(rc):
                rsl = slice(rc * 128, (rc + 1) * 128)
                s_ps = mmps.tile([128, T], F32, tag="mm", name="s_ps")
                for th in range(2):
                    psl = slice(th * 512, (th + 1) * 512)
                    tsl = slice(R + th * 512, R + (th + 1) * 512)
                    nc.tensor.matmul(
                        s_ps[:, psl], f_sb[0][:, rsl], f_sb[0][:, tsl],
                        start=True, stop=False,
                    )
                    nc.tensor.matmul(
                        s_ps[:, psl], f_sb[1][:, rsl], f_sb[1][:, tsl],
                        start=False, stop=True,
                    )
                e_sb = epool.tile([128, T], F32R, tag="e", name="e_sb")
                if rc == RC - 1:
                    # split the last exp so the tail drains per 512-col half
                    for th in range(2):
                        psl = slice(th * 512, (th + 1) * 512)
                        nc.scalar.activation(
                            e_sb[:, psl], s_ps[:, psl],
                            mybir.ActivationFunctionType.Exp,
                            bias=shift_sb, scale=1.0,
                        )
                else:
                    nc.scalar.activation(
                        e_sb, s_ps, mybir.ActivationFunctionType.Exp,
                        bias=shift_sb, scale=1.0,
                    )
                e_tiles[rc] = e_sb

            def pred_part(rc):
                e_sb = e_tiles.pop(rc)
                for th in range(2):
                    psl = slice(th * 512, (th + 1) * 512)
                    nc.tensor.matmul(
                        pred_ps[:, psl],
                        lab_sb[:, rc, :],
                        e_sb[:, psl],
                        start=(rc == 0), stop=(rc == RC - 1),
                    )

            # conv blocks feed S chunks; pred lags two chunks behind its exp
            PRED_LAG = 2
            emitted = []

            def emit_s(rc):
                s_part(rc)
                emitted.append(rc)
                if len(emitted) > PRED_LAG:
                    pred_part(emitted[len(emitted) - 1 - PRED_LAG])

            conv_block(3, act_cast=True)
            conv_block(0, act_cast=True)
            for rc in range(0, 8):
                emit_s(rc)
            conv_block(1, act_cast=False)
            for rc in range(8, 16):
                emit_s(rc)
            conv_block(2, act_cast=False)
            for rc in range(16, 24):
                emit_s(rc)
            for rc in emitted[-PRED_LAG:]:
                pred_part(rc)

            # ---- 5. normalize label rows by replicated denom rows ----
            # per 512-col half so the h0 DMA-out overlaps the h1 compute
            # (gpsimd cannot read PSUM, so both ops stay on DVE)
            rec = opool.tile([K_LAB, T], F32, tag="rec")
            o_sb = opool.tile([K_LAB, T], F32, tag="o")
            for th in range(2):
                psl = slice(th * 512, (th + 1) * 512)
                nc.vector.reciprocal_approx_fast(
                    rec[:, psl], pred_ps[0:K_LAB, psl]
                )
                nc.vector.tensor_mul(
                    o_sb[:, psl], pred_ps[32:32 + K_LAB, psl], rec[:, psl]
                )
                nc.sync.dma_start(out=out_d.ap()[:, psl], in_=o_sb[:, psl])

    nc.compile()
    return nc


_NC_CACHE = None


def _get_nc():
    global _NC_CACHE
    if _NC_CACHE is None:
        _NC_CACHE = _build_nc()
    return _NC_CACHE


def prep_in_maps(reference_images, target_images, reference_labels, w_feat):
    """Host-side sharding + layout prep (no arithmetic)."""
    ri = np.ascontiguousarray(reference_images, dtype=np.float32)
    ti = np.ascontiguousarray(target_images, dtype=np.float32)
    lab = np.ascontiguousarray(reference_labels, dtype=np.float32)
    wf = np.ascontiguousarray(w_feat, dtype=np.float32)

    w2 = np.zeros((KPAD, FEAT), ml_dtypes.bfloat16)
    w2[:KPIX] = wf.reshape(KPIX, FEAT).astype(ml_dtypes.bfloat16)
    imgs = np.concatenate([ri, ti], axis=1)  # [N, 4, H, W, C]
    # patchesT[n] : [(dy dx ch), (img py px)]
    ptT = np.ascontiguousarray(
        imgs.reshape(N, NIMG, HP, PATCH, HP, PATCH, C)
        .transpose(0, 3, 5, 6, 1, 2, 4)
        .reshape(N, KPIX, NPAT)
        .astype(ml_dtypes.bfloat16)
    )
    # aug labels: [128, RC, 48] = [ones16 | zeros16 | lab16] per r-chunk
    lab_sw = np.zeros((N, 128, RC, LABC), np.float32)
    lab_sw[:, :, :, 0:K_LAB] = 1.0
    lab_sw[:, :, :, 32:48] = (
        lab.reshape(N, RC, 128, K_LAB).transpose(0, 2, 1, 3)
    )
    lab_sw = np.ascontiguousarray(lab_sw.reshape(N, 128, RC * LABC))
    return [
        {"pt": ptT[n], "w": w2, "lab": lab_sw[n]} for n in range(N)
    ]


def run(in_maps, **kwargs):
    nc = _get_nc()
    return run_bass_kernel_spmd(nc, in_maps, list(range(N)), **kwargs)


def kernel(reference_images, target_images, reference_labels, w_feat):
    in_maps = prep_in_maps(
        reference_images, target_images, reference_labels, w_feat
    )
    res = run(in_maps)
    # device emits [16, T]; transpose to [T, 16] here (pure layout)
    out = np.stack(
        [np.ascontiguousarray(res.results[n]["out"].T) for n in range(N)]
    )
    return out.reshape(N, T_T, HP, HP, K_LAB)
